# revision 20
# baseline (speedup 1.0000x reference)
"""Trainium2 Bass kernel for BipartiteKernel GNN message passing.

  rel    = pos_query[dst] - pos_obs[src]            [E, 3]
  hid    = relu(rel @ W1 + b1)                      [E, 128]
  logits = hid @ W2 (+ b2: dropped, cancels in softmax)   [E, 4]
  v      = h_obs @ Wv + bv                          [N_o, 4, 32]
  attn   = softmax over edges grouped by dst        [E, 4]
  out    = segment_sum(attn * v[src], dst)          [N_q, 128]

Strategy (8 cores):
  - Edges sharded by contiguous dst range (6250 queries/core).
  - v-table built distributed (12544 rows/core) in bf16 with bv folded in
    (sum S*ex*(v+bv) = U + s*bv, so out = U'/s includes bv; empty queries
    give 0, matching the reference), one AllGather.
  - Max-free softmax: U'[q] = sum(exp(logit)*(v+bv)[src]), s[q] = sum(exp),
    out = U'/s in one pass over edges.
  - Edge MLP via stacked [W1; -W1] matmul on [posq; poso] streams, hid^T
    layout; logits via per-tile lhsT=hid^T matmul (ap_size=4: cheap).
  - pos6 uploaded transposed, loaded via transpose-DMA.
  - Per-edge v rows fetched with dma_gather (int16 idx, 4 table chunks).
  - Segment sum: ONE selector one-hot per 128-edge tile over a shared
    32-aligned SWIN-query window (dst-sorted tiles span ~30 queries across
    all cores), then 32-partition-stripe matmuls accumulate into the
    per-block PSUM region. S-builds split between DVE and gpsimd.
"""

import numpy as np
import ml_dtypes

import concourse.bacc as bacc
import concourse.mybir as mybir
from concourse.bass_utils import run_bass_kernel_spmd
from concourse.tile import TileContext

bf16 = ml_dtypes.bfloat16
P = 128
NCORES = 8
HEADS = 4
HD = 32
LATENT = 128
NQW = 128           # queries per output block (PSUM col region)
GBLK = 3            # blocks per supergroup (one PSUM bank: 3*132 <= 512)
TG = 8              # tiles per weight-op group
VG = 8              # obs tiles per v-build group
HG = 8              # tiles per hid/relu group
LG = 128            # tiles per logits/exp psum group
PG = 16             # tiles per pos-DMA chunk
WROW = 132          # w row: 128 weighted-v cols + 4 ex cols
F32 = mybir.dt.float32
BF16 = mybir.dt.bfloat16
I16 = mybir.dt.int16
DUMMY_DR = 100000.0


def _wrap_idx(lin):
    """[n] linear gather order -> [128, n/16] wrapped + 8x replicated."""
    n = len(lin)
    assert n % 16 == 0
    w = lin.reshape(n // 16, 16).T.astype(np.int16)
    return np.tile(w, (8, 1))


def _build_host(h_obs, pos_obs, pos_query, src, dst, Wv, bv, W1, b1, W2, b2):
    N_O = h_obs.shape[0]
    N_Q = pos_query.shape[0]
    assert N_Q % NCORES == 0 and N_O % NCORES == 0
    NQ_CORE = N_Q // NCORES
    NO_SLICE = N_O // NCORES
    NO_PAD = ((NO_SLICE + P - 1) // P) * P
    NROWS = NO_PAD * NCORES
    NCHUNK = 1
    while NROWS // NCHUNK > 32000:   # int16 gather index range
        NCHUNK *= 2
    assert NROWS % NCHUNK == 0
    CH = NROWS // NCHUNK
    NBLK = (NQ_CORE + NQW - 1) // NQW
    NSG = (NBLK + GBLK - 1) // GBLK
    QSG = NQW * GBLK                 # queries per supergroup

    # table rows: core-major (single AllGather output order)
    tbl_row = (src // NO_SLICE) * NO_PAD + (src % NO_SLICE)
    chunk_of = tbl_row // CH

    order = np.argsort(dst, kind="stable")
    dst_s = dst[order]
    core_bounds = np.searchsorted(dst_s, np.arange(NCORES + 1) * NQ_CORE)

    # per (core, sg, chunk) edge index lists (dst-sorted within)
    per = [[[None] * NCHUNK for _ in range(NSG)] for _ in range(NCORES)]
    for c in range(NCORES):
        eidx = order[core_bounds[c]:core_bounds[c + 1]]
        d_loc = dst[eidx] - c * NQ_CORE
        sg_of = d_loc // QSG
        ch = chunk_of[eidx]
        for sg in range(NSG):
            in_sg = sg_of == sg
            for cc in range(NCHUNK):
                sel = in_sg & (ch == cc)
                per[c][sg][cc] = eidx[sel]

    # shared run sizes (tiles), maxed over cores
    T_run = [[0] * NCHUNK for _ in range(NSG)]
    for sg in range(NSG):
        for cc in range(NCHUNK):
            mx = max(len(per[c][sg][cc]) for c in range(NCORES))
            T_run[sg][cc] = (mx + P - 1) // P

    sg_ntiles = [sum(T_run[sg]) for sg in range(NSG)]
    NT = sum(sg_ntiles)
    NSLOT = NT * P

    # pass 1: per (core, tile-slot) sg-local query min/max -> shared window
    tmin = np.full(NT, 1 << 30, np.int64)
    tmax = np.full(NT, -1, np.int64)
    for c in range(NCORES):
        t_base = 0
        for sg in range(NSG):
            run_off = 0
            for cc in range(NCHUNK):
                e = per[c][sg][cc]
                n = len(e)
                ntile = T_run[sg][cc]
                if ntile == 0:
                    continue
                d_sg = (dst[e] - c * NQ_CORE) - sg * QSG
                for tt in range(ntile):
                    gt = t_base + (run_off // P) + tt
                    lo, hi = tt * P, min((tt + 1) * P, n)
                    if lo < n:
                        dseg = d_sg[lo:hi]
                        tmin[gt] = min(tmin[gt], int(dseg.min()))
                        tmax[gt] = max(tmax[gt], int(dseg.max()))
                run_off += ntile * P
            t_base += sg_ntiles[sg]

    wbase = np.where(tmax >= 0, 32 * (tmin // 32), 0)
    span = int(np.where(tmax >= 0, tmax - wbase + 1, 0).max())
    SWIN = 64 if span <= 64 else (96 if span <= 96 else 128)
    assert span <= 128, f"tile query span {span} exceeds SWIN=128"

    pos6 = np.zeros((NCORES, NSLOT, 32), np.float32)   # transposed upload
    drel = np.full((NCORES, P, NT), DUMMY_DR, np.float32)
    idx16 = np.zeros((NCORES, P, NSLOT // 16), np.int16)
    # union specs per tile-slot: list of (block, poff, c0) 32-wide stripes
    spec_sets = [set() for _ in range(NT)]

    pq = pos_query.astype(np.float32)
    po = pos_obs.astype(np.float32)

    for c in range(NCORES):
        t_base = 0
        for sg in range(NSG):
            s_base = t_base * P
            run_off = 0
            for cc in range(NCHUNK):
                e = per[c][sg][cc]
                n = len(e)
                ntile = T_run[sg][cc]
                if ntile == 0:
                    continue
                nslots = ntile * P
                s0 = s_base + run_off
                if n > 0:
                    pos6[c, s0:s0 + n, 0:3] = pq[dst[e]]
                    pos6[c, s0:s0 + n, 3:6] = po[src[e]]
                lin = np.zeros(nslots, np.int64)
                lin[:n] = tbl_row[e] - cc * CH
                idx16[c, :, s0 // 16:(s0 + nslots) // 16] = _wrap_idx(lin)
                d_sg = (dst[e] - c * NQ_CORE) - sg * QSG   # [0, QSG)
                for tt in range(ntile):
                    gt = t_base + (run_off // P) + tt
                    lo, hi = tt * P, min((tt + 1) * P, n)
                    if lo < n:
                        dseg = d_sg[lo:hi]
                        w = int(wbase[gt])
                        drel[c, 0:hi - lo, gt] = dseg - w
                        for st in np.unique(dseg // 32).tolist():
                            spec_sets[gt].add(int(st))
                run_off += nslots
            t_base += sg_ntiles[sg]

    # merge each tile's touched 32-query stripes into maximal legal matmul
    # pieces (partition-offset rule: poff 0 allows width<=128, 64 allows
    # <=64, 32/96 allow 32). Specs: (block, poff, c0, width).
    tile_specs = []
    for gt in range(NT):
        stripes = sorted(spec_sets[gt])
        w = int(wbase[gt])
        pieces = []
        i = 0
        while i < len(stripes):
            st = stripes[i]
            b, poff = st // 4, (st % 4) * 32
            maxw = 4 if poff == 0 else (2 if poff == 64 else 1)
            # absorb any further stripes within reach (gaps add zeros)
            j = i + 1
            while j < len(stripes) and stripes[j] - st < maxw:
                j += 1
            wd = (stripes[j - 1] - st + 1) * 32
            pieces.append((b, poff, st * 32 - w, wd))
            i = j
        tile_specs.append(pieces)
    tile_w = [max(16, max((c0 + wd for (_b, _p, c0, wd) in tile_specs[gt]),
                          default=16)) for gt in range(NT)]

    # repack pos6 into transpose-DMA layout [NSLOT/4, 128]: per pos-chunk of
    # pn*128 slots, 4 streams of M=pn*32 slots; row j col 32u+k = comp k of
    # chunk slot u*M+j. The device XBAR-transposes each chunk to [128, M]
    # and the 4 streams land on partition blocks {0,32,64,96}.
    pos6t = np.zeros((NCORES, NSLOT // 4, P), np.float32)
    t_base = 0
    for sg in range(NSG):
        T_sg = sg_ntiles[sg]
        s_base = t_base * P
        for lg0 in range(0, T_sg, LG):
            lgn = min(LG, T_sg - lg0)
            for p0 in range(lg0, lg0 + lgn, PG):
                pn = min(PG, lg0 + lgn - p0)
                cs = s_base + p0 * P
                M = pn * 32
                blk = pos6[:, cs:cs + pn * P, :].reshape(NCORES, 4, M, 32)
                pos6t[:, cs // 4:cs // 4 + M, :] = (
                    blk.transpose(0, 2, 1, 3).reshape(NCORES, M, P))
        t_base += T_sg

    j01 = np.broadcast_to(
        np.arange(SWIN, dtype=np.float32), (P, SWIN)).astype(bf16)
    w1s1 = np.zeros((32, LATENT), np.float32)  # K rounds up to 32 on the PE
    w1s1[0:3] = W1
    w1s1[3:6] = -W1
    # stacked 4x so pos-streams 0-2 (base partitions 0/32/64) have a
    # matching lhsT; stream 3 (base 96 is illegal) uses w1sz: a K=64 lhsT
    # at base 64 whose top half is zero, so stream 2's partitions drop out
    w1s = np.tile(w1s1, (4, 1)).astype(bf16)
    w1sz = np.zeros((P, LATENT), np.float32)
    w1sz[96:128] = w1s1
    w1sz = w1sz.astype(bf16)

    host = {
        "NQ_CORE": NQ_CORE, "NO_SLICE": NO_SLICE, "NO_PAD": NO_PAD,
        "NROWS": NROWS, "NCHUNK": NCHUNK, "CH": CH, "NBLK": NBLK,
        "NSG": NSG, "NT": NT, "NSLOT": NSLOT, "T_run": T_run,
        "sg_ntiles": sg_ntiles, "tile_specs": tile_specs, "SWIN": SWIN,
        "tile_w": tile_w,
    }
    in_maps = []
    for c in range(NCORES):
        h_sl = np.zeros((LATENT, NO_PAD), np.float32)
        h_sl[:, :NO_SLICE] = h_obs[c * NO_SLICE:(c + 1) * NO_SLICE].T
        in_maps.append({
            "hslT": np.ascontiguousarray(h_sl.astype(bf16)),
            "pos6t": np.ascontiguousarray(pos6t[c].astype(bf16)),
            "dr0": np.ascontiguousarray(drel[c]),
            "idx16": np.ascontiguousarray(idx16[c]),
            "j01": np.ascontiguousarray(j01),
            "w1s": np.ascontiguousarray(w1s),
            "w1sz": np.ascontiguousarray(w1sz),
            "wv": np.ascontiguousarray(
                Wv.astype(np.float32).reshape(LATENT, HEADS, HD)
                .transpose(0, 2, 1).reshape(LATENT, LATENT).astype(bf16)),
            "w2": np.ascontiguousarray(W2.astype(np.float32).astype(bf16)),
            "b1": np.ascontiguousarray(b1.astype(np.float32).reshape(LATENT, 1)),
            "bv": np.ascontiguousarray(np.broadcast_to(
                bv.astype(np.float32).reshape(HEADS, HD).T.reshape(LATENT),
                (P, LATENT)).copy()),
            "bvrow": np.ascontiguousarray(np.tile(
                bv.astype(np.float32).reshape(HEADS, HD).T.reshape(LATENT),
                4)[None, :].astype(bf16)),
        })
    return host, in_maps


def _build_nc(host):
    NQ_CORE = host["NQ_CORE"]
    NO_PAD = host["NO_PAD"]
    NROWS = host["NROWS"]
    NCHUNK = host["NCHUNK"]
    CH = host["CH"]
    NBLK = host["NBLK"]
    NSG = host["NSG"]
    NT = host["NT"]
    NSLOT = host["NSLOT"]
    T_run = host["T_run"]
    sg_ntiles = host["sg_ntiles"]
    tile_specs = host["tile_specs"]
    SWIN = host["SWIN"]
    tile_w = host["tile_w"]

    nc = bacc.Bacc()
    hslT = nc.dram_tensor("hslT", [LATENT, NO_PAD], BF16, kind="ExternalInput")
    pos6t = nc.dram_tensor("pos6t", [NSLOT // 4, P], BF16, kind="ExternalInput")
    dr0 = nc.dram_tensor("dr0", [P, NT], F32, kind="ExternalInput")
    idx16 = nc.dram_tensor("idx16", [P, NSLOT // 16], I16, kind="ExternalInput")
    j01 = nc.dram_tensor("j01", [P, SWIN], BF16, kind="ExternalInput")
    w1s = nc.dram_tensor("w1s", [P, LATENT], BF16, kind="ExternalInput")
    w1sz = nc.dram_tensor("w1sz", [P, LATENT], BF16, kind="ExternalInput")
    wv = nc.dram_tensor("wv", [LATENT, LATENT], BF16, kind="ExternalInput")
    w2 = nc.dram_tensor("w2", [LATENT, HEADS], BF16, kind="ExternalInput")
    b1 = nc.dram_tensor("b1", [LATENT, 1], F32, kind="ExternalInput")
    bv = nc.dram_tensor("bv", [P, LATENT], F32, kind="ExternalInput")
    bvrow = nc.dram_tensor("bvrow", [1, 512], BF16, kind="ExternalInput")
    out_q = nc.dram_tensor("out_q", [NQ_CORE, LATENT], F32, kind="ExternalOutput")
    # f32-typed bf16-pair table: the SWDGE gather's engine cost scales with
    # element count, so 4-byte elements halve it (bytes are identical;
    # int64 would halve again but the PJRT execute path rejects it)
    vslice = nc.dram_tensor("vslice", [NO_PAD, LATENT // 2], F32)
    vtable = nc.dram_tensor("vtable", [NROWS, LATENT // 2], F32,
                            addr_space="Shared")

    Relu = mybir.ActivationFunctionType.Relu
    Exp = mybir.ActivationFunctionType.Exp
    ALU = mybir.AluOpType

    # balance S-builds between DVE and gpsimd (gpsimd is lighter once the
    # gather runs at f32-element cost)
    s_rr = [0]
    relu_n = [0]

    def s_engine(sg):
        s_rr[0] += 1
        dve_share = 1 if sg >= 11 else 3
        return nc.vector if (s_rr[0] % 8) < dve_share else nc.gpsimd

    with TileContext(nc) as tc:
        with tc.tile_pool(name="const", bufs=1) as cpool:
            # 8 interleaved chunks across SP and gpsimd so the first
            # v-build matmul starts ~1us in instead of waiting 4.8us
            HCH = (NO_PAD + 7) // 8
            hslT_sb = cpool.tile([LATENT, NO_PAD], BF16)
            for hi_ in range(8):
                lo = hi_ * HCH
                hi2 = min(NO_PAD, lo + HCH)
                if lo >= hi2:
                    break
                heng = nc.sync if hi_ % 2 == 0 else nc.gpsimd
                heng.dma_start(out=hslT_sb[:, lo:hi2], in_=hslT[:, lo:hi2])
            wv_sb = cpool.tile([LATENT, LATENT], BF16)
            nc.scalar.dma_start(out=wv_sb, in_=wv[:, :])
            bv_sb = cpool.tile([P, LATENT], F32)
            nc.scalar.dma_start(out=bv_sb, in_=bv[:, :])
            bvrow_sb = cpool.tile([1, 512], BF16)
            nc.scalar.dma_start(out=bvrow_sb, in_=bvrow[:, :])
            one_sb = cpool.tile([1, P], BF16)
            nc.vector.memset(one_sb, 1.0)
            w1s_sb = cpool.tile([P, LATENT], BF16)
            nc.scalar.dma_start(out=w1s_sb, in_=w1s[:, :])
            w1sz_sb = cpool.tile([P, LATENT], BF16)
            nc.scalar.dma_start(out=w1sz_sb, in_=w1sz[:, :])
            w2_sb = cpool.tile([LATENT, HEADS], BF16)
            nc.scalar.dma_start(out=w2_sb, in_=w2[:, :])
            b1_sb = cpool.tile([LATENT, 1], F32)
            nc.scalar.dma_start(out=b1_sb, in_=b1[:, :])
            j01_sb = cpool.tile([P, SWIN], BF16)
            nc.scalar.dma_start(out=j01_sb, in_=j01[:, :])
            # idx/dr on gpsimd pre-collective (Pool is idle then), so
            # the first gather can fire the moment the AllGather lands
            idx_sb = cpool.tile([P, NSLOT // 16], I16)
            dr_sb = cpool.tile([P, NT], F32)
            nc.gpsimd.dma_start(out=idx_sb, in_=idx16[:, :])
            nc.gpsimd.dma_start(out=dr_sb, in_=dr0[:, :])
            zer_sb = cpool.tile([P, 512], BF16)
            nc.vector.memset(zer_sb, 0.0)

            with (
                tc.tile_pool(name="pos", bufs=3) as pos_pool,
                tc.tile_pool(name="hid", bufs=4) as hid_pool,
                tc.tile_pool(name="ex", bufs=NSG + 1) as ex_pool,
                tc.tile_pool(name="vg", bufs=6) as vg_pool,
                tc.tile_pool(name="wt", bufs=4) as w_pool,
                tc.tile_pool(name="s", bufs=8) as s_pool,
                tc.tile_pool(name="outp", bufs=6) as out_pool,
                tc.tile_pool(name="hps", bufs=2, space="PSUM") as hps_pool,
                tc.tile_pool(name="lps", bufs=2, space="PSUM") as lps_pool,
                tc.tile_pool(name="ups", bufs=2, space="PSUM") as ups_pool,
            ):
                # ---- phase 1: v slice build (+bv) + one AllGather ----
                # PSUM from the ups pool (idle until phase C) so the MLP's
                # hps buffers are free from the start
                nvt = NO_PAD // P
                VB = 4
                for i0 in range(0, nvt, VB):
                    gn = min(VB, nvt - i0)
                    v_ps = ups_pool.tile([P, 512], F32, tag="u")
                    for t in range(gn):
                        nc.tensor.matmul(
                            v_ps[:, t * LATENT:(t + 1) * LATENT],
                            hslT_sb[:, (i0 + t) * P:(i0 + t + 1) * P], wv_sb,
                            start=t == 0, stop=t == gn - 1)
                    vstage = out_pool.tile([P, VB * LATENT], BF16, tag="vstage")
                    nc.vector.tensor_tensor(
                        out=vstage[:, 0:gn * LATENT].rearrange(
                            "p (t c) -> p t c", c=LATENT),
                        in0=v_ps[:, 0:gn * LATENT].rearrange(
                            "p (t c) -> p t c", c=LATENT),
                        in1=bv_sb[:, :].to_broadcast(
                            [P, LATENT, gn]).rearrange("p c t -> p t c"),
                        op=ALU.add)
                    # alternate gpsimd/SP issue (both have slack here);
                    # splitting halves the serial store time gating the AG
                    eng = nc.gpsimd if (i0 // VB) % 2 else nc.sync
                    eng.dma_start(
                        out=vslice[i0 * P:(i0 + gn) * P, :].bitcast(
                            BF16).rearrange("(t p) c -> p t c", p=P),
                        in_=vstage[:, 0:gn * LATENT].rearrange(
                            "p (t c) -> p t c", c=LATENT))
                nc.gpsimd.collective_compute(
                    "AllGather", ALU.bypass,
                    replica_groups=[list(range(NCORES))],
                    ins=[vslice[:, :]],
                    outs=[vtable[:, :]],
                )

                # ==== phase B: MLP (hid^T -> logits -> ex) for ALL sgs ====
                # so the PE never interleaves MLP with selector matmuls
                # (which wait on gathers) and Act's relu latency is hidden
                # by a one-chunk logits skid.
                ex_sgs = {}
                pend = []   # deferred logits: (lg_ps, lgoff, hsb, pn)
                t_base = 0
                for sg in range(NSG):
                    T_sg = sg_ntiles[sg]
                    if T_sg == 0:
                        continue
                    s_base = t_base * P
                    ex_sg = ex_pool.tile([P, T_sg * HEADS], BF16, tag="ex")
                    ex_sgs[sg] = ex_sg
                    for lg0 in range(0, T_sg, LG):
                        lgn = min(LG, T_sg - lg0)
                        lg_ps = lps_pool.tile([P, LG * HEADS], F32, tag="lg")
                        for p0 in range(lg0, lg0 + lgn, PG):
                            pn = min(PG, lg0 + lgn - p0)
                            M = pn * 32
                            r0 = (s_base + p0 * P) // 4
                            ps = pos_pool.tile([P, PG * 32], BF16, tag="pos")
                            nc.sync.dma_start(
                                out=ps[0:P, 0:M],
                                in_=pos6t[r0:r0 + M, :],
                                transpose=True)
                            # chunk-wide hid^T in bf16; 4 slot-streams on
                            # partition blocks of ps, 2 PSUM halves with the
                            # second stream bank-aligned at col 512
                            hsb = hid_pool.tile([P, PG * P], BF16, tag="hsb")
                            for half in range(2):
                                hid_ps = hps_pool.tile([P, HG * P], F32, tag="hid")
                                for uu in range(2):
                                    u = half * 2 + uu
                                    if u < 3:
                                        nc.tensor.matmul(
                                            hid_ps[:, uu * 512:uu * 512 + M],
                                            w1s_sb[u * 32:(u + 1) * 32, :],
                                            ps[u * 32:(u + 1) * 32, 0:M],
                                            start=True, stop=True)
                                    else:
                                        nc.tensor.matmul(
                                            hid_ps[:, uu * 512:uu * 512 + M],
                                            w1sz_sb[64:128, :],
                                            ps[64:128, 0:M],
                                            start=True, stop=True)
                                relu_n[0] += 1
                                oview = hsb[:, half * 2 * M:(half + 1) * 2 * M] \
                                    .rearrange("p (u c) -> p u c", c=M)
                                iview = hid_ps[:, :].rearrange(
                                    "p (u c) -> p u c", c=512)[:, :, 0:M]
                                if 12 <= relu_n[0] < 28 and relu_n[0] % 2 == 0:
                                    nc.vector.tensor_scalar(
                                        out=oview, in0=iview,
                                        scalar1=b1_sb[:, 0:1], scalar2=0.0,
                                        op0=ALU.add, op1=ALU.max)
                                else:
                                    nc.scalar.activation(
                                        oview, iview,
                                        Relu, bias=b1_sb[:, 0:1], scale=1.0)
                            while len(pend) >= 2:
                                plg, poff_, phsb, ppn = pend.pop(0)
                                for t in range(ppn):
                                    nc.tensor.matmul(
                                        plg[:, (poff_ + t) * HEADS:
                                            (poff_ + t + 1) * HEADS],
                                        phsb[:, t * P:(t + 1) * P], w2_sb,
                                        start=True, stop=True)
                            pend.append((lg_ps, p0 - lg0, hsb, pn))
                        # drain before exp reads lg_ps
                        while pend:
                            plg, poff_, phsb, ppn = pend.pop(0)
                            for t in range(ppn):
                                nc.tensor.matmul(
                                    plg[:, (poff_ + t) * HEADS:
                                        (poff_ + t + 1) * HEADS],
                                    phsb[:, t * P:(t + 1) * P], w2_sb,
                                    start=True, stop=True)
                        nc.scalar.activation(
                            ex_sg[:, lg0 * HEADS:(lg0 + lgn) * HEADS],
                            lg_ps[:, 0:lgn * HEADS], Exp)
                    t_base += T_sg

                # ==== phase C: gather, weight, selector, normalize ====
                t_base = 0
                for sg in range(NSG):
                    T_sg = sg_ntiles[sg]
                    if T_sg == 0:
                        continue
                    s_base = t_base * P
                    ex_sg = ex_sgs[sg]

                    # ---- gather v rows per chunk run ----
                    vg_runs = {}
                    run_t0 = 0
                    for cc in range(NCHUNK):
                        ntile = T_run[sg][cc]
                        if ntile == 0:
                            continue
                        vg = vg_pool.tile([P, ntile * LATENT], BF16, tag="vg")
                        # SWDGE descriptor carveout holds 1024 descriptors;
                        # larger dma_gather calls hang the ucode.
                        for g0 in range(0, ntile, 8):
                            gtile = min(8, ntile - g0)
                            s0 = s_base + (run_t0 + g0) * P
                            nidx = gtile * P
                            nc.gpsimd.dma_gather(
                                vg[:, g0 * LATENT:(g0 + gtile) * LATENT]
                                .bitcast(F32).rearrange(
                                    "p (t c) -> p t c", c=LATENT // 2),
                                vtable[cc * CH:(cc + 1) * CH, :],
                                idx_sb[:, s0 // 16:(s0 + nidx) // 16],
                                nidx, nidx, LATENT // 2,
                            )
                        vg_runs[cc] = (vg, run_t0, ntile)
                        run_t0 += ntile

                    # ---- weight + selector matmuls ----
                    # A zeroing matmul writes the ENTIRE bank first; the data
                    # matmuls then accumulate onto it in any order (their S
                    # inputs come from two engines, so completion order is
                    # not program order) with a true RAW dependency enforcing
                    # the init-first order. skip_group_check silences the
                    # sim's one-group-per-bank formalism.
                    u_ps = ups_pool.tile([P, 512], F32, tag="u")
                    ucol = lambda b: b * WROW
                    nc.tensor.matmul(
                        u_ps[:, 0:GBLK * WROW], zer_sb[:, 0:128],
                        zer_sb[:, 0:GBLK * WROW], start=True, stop=True)
                    for cc in range(NCHUNK):
                        if cc not in vg_runs:
                            continue
                        vg, run_t0, ntile = vg_runs[cc]
                        for w0 in range(0, ntile, TG):
                            wn = min(TG, ntile - w0)
                            wt = w_pool.tile([P, TG * WROW], BF16, tag="w")
                            wt_v = wt[:, 0:wn * WROW].rearrange(
                                "p (t c) -> p t c", c=WROW)
                            vg_v = vg[:, w0 * LATENT:(w0 + wn) * LATENT].rearrange(
                                "p (t c) -> p t c", c=LATENT)
                            exsl = ex_sg[:, (run_t0 + w0) * HEADS:
                                         (run_t0 + w0 + wn) * HEADS].rearrange(
                                "p (t h) -> p t h", h=HEADS)
                            # v-table columns are d-major (col = d*4+h): the
                            # ex broadcast has step-1 innermost (h) -> DVE 2x
                            nc.vector.tensor_tensor(
                                out=wt_v[:, :, 0:LATENT].rearrange(
                                    "p t (d h) -> p t d h", h=HEADS),
                                in0=vg_v.rearrange("p t (d h) -> p t d h", h=HEADS),
                                in1=exsl.to_broadcast([P, wn, HEADS, HD]).rearrange(
                                    "p t h d -> p t d h"),
                                op=ALU.mult)
                            nc.gpsimd.tensor_copy(
                                out=wt_v[:, :, LATENT:WROW], in_=exsl)
                            for t in range(wn):
                                tl = run_t0 + w0 + t
                                gt = t_base + tl
                                specs = tile_specs[gt]
                                if not specs:
                                    continue
                                s_sb = s_pool.tile([P, SWIN], BF16, tag="s")
                                tw = tile_w[gt]
                                s_engine(sg).tensor_scalar(
                                    out=s_sb[:, 0:tw], in0=j01_sb[:, 0:tw],
                                    scalar1=dr_sb[:, gt:gt + 1],
                                    scalar2=None, op0=ALU.is_equal)
                                for b, poff, c0, wd in specs:
                                    nc.tensor.matmul(
                                        u_ps[poff:poff + wd,
                                             ucol(b):ucol(b) + WROW],
                                        s_sb[:, c0:c0 + wd],
                                        wt[:, t * WROW:(t + 1) * WROW],
                                        start=False, stop=False,
                                        skip_group_check=True,
                                        tile_position=(0, poff))

                    # ---- normalize + store (consolidated per supergroup) ----
                    nb = min(GBLK, NBLK - sg * GBLK)
                    full_nb = sum(
                        1 for b in range(nb)
                        if min(NQW, NQ_CORE - (sg * GBLK + b) * NQW) == NQW)
                    ostage = out_pool.tile([P, GBLK * LATENT], F32, tag="ostage")
                    u_v = u_ps[:, 0:nb * WROW].rearrange(
                        "p (b c) -> p b c", c=WROW)
                    stmp = out_pool.tile([P, GBLK * HEADS], F32, tag="stmp")
                    nc.vector.tensor_scalar(
                        out=stmp[:, 0:nb * HEADS].rearrange(
                            "p (b h) -> p b h", h=HEADS),
                        in0=u_v[:, :, LATENT:WROW],
                        scalar1=1e-30, scalar2=None, op0=ALU.add)
                    rcp = out_pool.tile([P, GBLK * HEADS], F32, tag="rcp")
                    nc.vector.reciprocal(rcp[:, 0:nb * HEADS],
                                         stmp[:, 0:nb * HEADS])
                    nc.vector.tensor_tensor(
                        out=ostage[:, 0:nb * LATENT].rearrange(
                            "p (b d h) -> p b d h", d=HD, h=HEADS),
                        in0=u_v[:, :, 0:LATENT].rearrange(
                            "p b (d h) -> p b d h", h=HEADS),
                        in1=rcp[:, 0:nb * HEADS].rearrange(
                            "p (b h) -> p b h", h=HEADS).to_broadcast(
                            [P, nb, HEADS, HD]).rearrange("p b h d -> p b d h"),
                        op=ALU.mult)
                    for b in range(nb):
                        blk = sg * GBLK + b
                        rows = min(NQW, NQ_CORE - blk * NQW)
                        if rows < NQW:
                            nc.sync.dma_start(
                                out=out_q[blk * NQW:blk * NQW + rows, :],
                                in_=ostage[0:rows, b * LATENT:(b + 1) * LATENT])
                    if full_nb > 0:
                        q0 = sg * GBLK * NQW
                        nc.sync.dma_start(
                            out=out_q[q0:q0 + full_nb * NQW, :].rearrange(
                                "(t p) c -> p t c", p=P),
                            in_=ostage[:, 0:full_nb * LATENT].rearrange(
                                "p (t c) -> p t c", c=LATENT))
                    t_base += T_sg

    nc.compile()
    return nc


def kernel(h_obs, pos_obs, pos_query, src, dst, Wv, bv, W1, b1, W2, b2,
           trace=False):
    h_obs = np.asarray(h_obs)
    pos_obs = np.asarray(pos_obs)
    pos_query = np.asarray(pos_query)
    src = np.asarray(src)
    dst = np.asarray(dst)
    host, in_maps = _build_host(
        h_obs, pos_obs, pos_query, src, dst,
        np.asarray(Wv), np.asarray(bv), np.asarray(W1), np.asarray(b1),
        np.asarray(W2), np.asarray(b2))
    nc = _build_nc(host)
    res = run_bass_kernel_spmd(nc, in_maps, core_ids=list(range(NCORES)),
                               trace=trace)
    out = np.concatenate(
        [np.asarray(res.results[c]["out_q"]) for c in range(NCORES)], axis=0)
    # device emits d-major feature columns (d*4+h); restore h-major
    out = np.ascontiguousarray(
        out.reshape(-1, HD, HEADS).transpose(0, 2, 1).reshape(-1, LATENT))
    if trace:
        return out.astype(np.float32), res
    return out.astype(np.float32)

